# revision 24
# baseline (speedup 1.0000x reference)
"""Trainium2 Bass kernel for the BiaffineLayer problem.

Math (per batch b):
  out[l, m, c] = x1[l] @ W1[c] + x2[m] @ W2[c]
              + sum_h x1[l,h] * x2[m,h] * W3[c,h]
              + sum_h |x1[l,h] - x2[m,h]| * W4[c,h] + bias[c]
  shapes: x1, x2 [2, 512, 128]; W [25, 512]; bias [25]; out [2, 512, 512, 25]

Sharding: 8 cores = 2 batches x 4 m-blocks of 128 columns.

Device computes ONLY the pairwise part, in bf16:
  out_dev[l, (m, c)] = sum_h x1t[h,l] * v3[h,(m,c)] + sum_h D_m[h,l] * w4t[h,c]
  with v3[h,(m,c)] = x2t[h,m]*W3T[h,c] (host-computed, DMA'd in) and
  D_m = relu(x1t - x2t[:,m]) (|d| = 2 relu(d) - d; the -d part is rank
  structured and folds into the host epilogue), w4t = (2 W4).T.

Host epilogue adds the rank-structured terms (cheap broadcasts):
  full = out_dev + x1 @ (W1 - W4).T [over l] + (x2 @ (W2 + W4).T + b) [over m]

Per-core schedule (4 blocks of 32 m's, software-pipelined one block ahead):
  - D tiles [128h, 512l] bf16: 7/32 shipped precomputed from host via the
    sync DMA queue; 20/32 on DVE (tensor_scalar sub+max, 2x mode at 341ns);
    5/32 on ACT (Relu with -x2 bias, 641ns).
  - per l-chunk: one PSUM double tile [128, 1024] f32 (two banks: cols
    0:400 and 512:912); 32 t4 matmuls (D_j l-slice stationary, w4t moving;
    start=True only on the first matmul of each bank - start zeroes the
    WHOLE 2KB bank) then 2 t3 matmuls (x1t chunk stationary, v3 slice
    moving, stop=True) - t4-first so the PE needn't wait for the v3 DMA.
  - ACT copies PSUM -> SBUF bf16 [128, 800] (strided 2-bank read), DMA out
    issued from the gpsimd sequencer (its engine is otherwise idle).
"""

import sys

sys.path.insert(0, "/opt/trn_rl_repo")

from contextlib import ExitStack

import numpy as np

import concourse.bass as bass
import concourse.tile as tile
from concourse import bacc, bass_utils, mybir

F32 = mybir.dt.float32
BF16 = mybir.dt.bfloat16

B, L, H, C = 2, 512, 128, 25
MB = 128            # m-block per core
N_CORES = 8
BLK = 32            # m's per psum double-block
N_BLK = MB // BLK   # 4
LCHUNK = 128
N_LC = L // LCHUNK  # 4
HBANK = 16 * C      # 400 used columns per psum bank
PSW = 1024          # psum double tile width (2 banks, f32)

# Engine assignment for the 32 D tiles of each block. The PE consumes js in
# ascending order, so early js go to the fastest producer (DVE at ~263ns
# sustained), ACT tiles (~715ns) are interleaved at its pace, and shipped
# tiles (arriving via batched DMA) fill the tail. Block 0 ships least (its
# tiles are needed before the DMA streams can deliver much); later blocks
# ship more since their deadlines are further out.
SHIP_BLK = {0: tuple(range(26, 32)), 1: tuple(range(20, 32)),
            2: tuple(range(20, 32)), 3: tuple(range(20, 32))}
ACT_BLK = {0: (2, 6, 10, 14, 18), 1: (2, 6, 10, 14),
           2: (2, 6, 10, 14), 3: (2, 6, 10, 14)}
NSHIP = sum(len(v) for v in SHIP_BLK.values())
SHIP_OFF = {}
_off = 0
for _k in range(N_BLK):
    SHIP_OFF[_k] = _off
    _off += len(SHIP_BLK[_k])
# which engine copies each (blk, lc) PSUM tile to SBUF (default ACT)
COPY_ENG = {k: {3: "dve"} for k in range(N_BLK)}


def build_kernel(nc: bass.Bass, repeat: int = 1):
    # x1t and w4t are packed into one tensor so the critical small loads are
    # a single DMA each; tiny multi-descriptor DMAs get starved behind the
    # big ship streams otherwise.
    hbf_d = nc.dram_tensor("hbf", (H, L + C), BF16, kind="ExternalInput").ap()
    x2t_d = nc.dram_tensor("x2t", (H, MB), F32, kind="ExternalInput").ap()
    v3_d = nc.dram_tensor("v3", (H, MB * C), BF16, kind="ExternalInput").ap()
    dship_d = nc.dram_tensor("dship", (H, NSHIP * L), BF16,
                             kind="ExternalInput").ap()
    out = nc.dram_tensor("out", (L, MB * C), BF16, kind="ExternalOutput").ap()

    with tile.TileContext(nc) as tc, ExitStack() as ctx:
      const = ctx.enter_context(tc.tile_pool(name="const", bufs=1))
      ship = ctx.enter_context(tc.tile_pool(name="ship", bufs=N_BLK))
      dpool = ctx.enter_context(tc.tile_pool(name="dpool", bufs=46))
      opool = ctx.enter_context(tc.tile_pool(name="opool", bufs=6))
      psum = ctx.enter_context(tc.tile_pool(name="psum", bufs=4, space="PSUM"))
      for _rep in range(repeat):
        # ---- critical small inputs first on the sync queue: x2t gates all
        # D-gen, hbf (x1t+w4t) gates DVE D-gen and every matmul ----
        x2t = const.tile([H, MB], F32)
        nc.sync.dma_start(x2t[:], x2t_d[:])
        hbf = const.tile([H, L + C], BF16)
        nc.sync.dma_start(hbf[:], hbf_d[:])
        x1t = hbf[:, 0:L]
        w4t = hbf[:, L:L + C]
        # negx2 (the ACT Relu bias) computed on-device: cheaper than a DMA
        negx2 = const.tile([H, MB], F32)
        nc.vector.tensor_scalar_mul(negx2[:], x2t[:], -1.0)

        # shipped D tiles: ONE batched DMA per block (per-tile issue is
        # ~680ns of DGE time, so per-tile DMAs would trickle in too slowly).
        # Block 0 + v3 on sync right after the small loads; blocks 1-3
        # stream on the gpsimd queue.
        ship_blk = {}

        def emit_ship(k, eng):
            w = len(SHIP_BLK[k]) * L
            st = ship.tile([H, w], BF16, tag="ship", name=f"ship_{k}")
            o = SHIP_OFF[k] * L
            eng.dma_start(st[:], dship_d[:, o:o + w])
            ship_blk[k] = st

        emit_ship(0, nc.sync)
        v3 = const.tile([H, MB * C], BF16)
        nc.sync.dma_start(v3[:], v3_d[:])
        emit_ship(1, nc.gpsimd)
        emit_ship(2, nc.gpsimd)
        emit_ship(3, nc.gpsimd)

        # ---- main loop, software-pipelined: D-gen one block ahead ----
        all_dts = {}

        def emit_dgen(k):
            ship_j, act_j = SHIP_BLK[k], ACT_BLK[k]
            dts = {}
            for j in range(BLK):
                if j in ship_j:
                    sj = ship_j.index(j)
                    dts[j] = (ship_blk[k], sj * L)
                    continue
                m = k * BLK + j
                dt_ = dpool.tile([H, L], BF16, tag="d")
                if j in act_j:
                    nc.scalar.activation(
                        dt_[:], x1t, mybir.ActivationFunctionType.Relu,
                        bias=negx2[:, m:m + 1], scale=1.0)
                else:
                    nc.vector.tensor_scalar(
                        dt_[:], x1t, x2t[:, m:m + 1], 0.0,
                        op0=mybir.AluOpType.subtract, op1=mybir.AluOpType.max)
                dts[j] = (dt_, 0)
            all_dts[k] = dts

        def emit_compute(k):
            dts = all_dts[k]
            for lc in range(N_LC):
                lsl = slice(lc * LCHUNK, (lc + 1) * LCHUNK)
                ps = psum.tile([LCHUNK, PSW], F32)
                for j in range(BLK):
                    col = j * C if j < 16 else 512 + (j - 16) * C
                    src, off = dts[j]
                    nc.tensor.matmul(
                        ps[:, col:col + C],
                        src[:, off + lc * LCHUNK:off + (lc + 1) * LCHUNK],
                        w4t,
                        start=(j % 16 == 0), stop=False,
                        skip_group_check=True)
                v0 = k * 2 * HBANK
                nc.tensor.matmul(
                    ps[:, 0:HBANK], x1t[:, lsl], v3[:, v0:v0 + HBANK],
                    start=False, stop=True, skip_group_check=True)
                nc.tensor.matmul(
                    ps[:, 512:512 + HBANK], x1t[:, lsl],
                    v3[:, v0 + HBANK:v0 + 2 * HBANK],
                    start=False, stop=True, skip_group_check=True)
                o_sb = opool.tile([LCHUNK, 2 * HBANK], BF16)
                ps_v = ps[:].rearrange("p (b c) -> p b c", b=2)[:, :, 0:HBANK]
                o_v = o_sb[:].rearrange("p (b c) -> p b c", b=2)
                ceng = COPY_ENG.get(k, {}).get(lc, "act")
                if ceng == "dve":
                    nc.vector.tensor_copy(o_v, ps_v)
                else:
                    nc.scalar.copy(o_v, ps_v)
                dma_eng = nc.sync if k < 2 else nc.gpsimd
                dma_eng.dma_start(
                    out[lsl, k * 2 * HBANK:(k + 1) * 2 * HBANK], o_sb[:])

        for k in range(N_BLK + 1):
            if k < N_BLK:
                emit_dgen(k)
            if k >= 1:
                emit_compute(k - 1)
    return nc


_COMPILED = {}


def _get_compiled():
    if "nc" not in _COMPILED:
        nc = bacc.Bacc("TRN2", target_bir_lowering=False, debug=False,
                       num_devices=N_CORES)
        build_kernel(nc)
        nc.compile()
        _COMPILED["nc"] = nc
    return _COMPILED["nc"]


def make_in_maps(x1, x2, W, b):
    W1, W2, W3, W4 = (W[:, 0:H], W[:, H:2 * H], W[:, 2 * H:3 * H],
                      W[:, 3 * H:4 * H])
    bf = np.dtype("bfloat16") if hasattr(np, "bfloat16") else None
    import jax.numpy as jnp  # bf16 conversion helper

    def to_bf16(a):
        return np.asarray(jnp.asarray(a, dtype=jnp.bfloat16))

    w4t = (2.0 * W4).T.astype(np.float32)                # [H, C]
    w3t = np.ascontiguousarray(W3.T, dtype=np.float32)   # [H, C]
    in_maps = []
    for cid in range(N_CORES):
        bb, mblk = cid // 4, cid % 4
        m0 = mblk * MB
        x1t = np.ascontiguousarray(x1[bb].T, dtype=np.float32)   # [H, L]
        x2t = np.ascontiguousarray(x2[bb, m0:m0 + MB].T, dtype=np.float32)
        v3 = (x2t[:, :, None] * w3t[:, None, :]).reshape(H, MB * C)
        ship_ms = [k * BLK + j for k in range(N_BLK) for j in SHIP_BLK[k]]
        dship = np.maximum(
            x1t[None, :, :] - x2t.T[ship_ms][:, :, None], 0.0)  # [NSHIP,H,L]
        dship = dship.transpose(1, 0, 2).reshape(H, NSHIP * L)
        in_maps.append({
            "hbf": to_bf16(np.concatenate([x1t, w4t], axis=1)),
            "x2t": x2t,
            "v3": to_bf16(v3),
            "dship": to_bf16(dship),
        })
    return in_maps


def run_on_device(x1, x2, W, b, trace=False, trace_kwargs=None):
    nc = _get_compiled()
    in_maps = make_in_maps(x1, x2, W, b)
    res = bass_utils.run_bass_kernel_spmd(
        nc, in_maps, core_ids=list(range(N_CORES)), trace=trace,
        **(trace_kwargs or {}))
    W1, W2, W4 = W[:, 0:H], W[:, H:2 * H], W[:, 3 * H:4 * H]
    t1 = x1 @ (W1 - W4).T                    # [B, L, C]
    t2 = x2 @ (W2 + W4).T + b                # [B, L, C]
    full = np.empty((B, L, L, C), dtype=np.float32)
    for cid in range(N_CORES):
        bb, mblk = cid // 4, cid % 4
        m0 = mblk * MB
        blkout = np.asarray(res.results[cid]["out"]).astype(
            np.float32).reshape(L, MB, C)
        blkout += t1[bb][:, None, :]
        blkout += t2[bb, m0:m0 + MB][None, :, :]
        full[bb, :, m0:m0 + MB, :] = blkout
    return full, res


def kernel(x1, x2, W, b):
    x1 = np.asarray(x1, dtype=np.float32)
    x2 = np.asarray(x2, dtype=np.float32)
    W = np.asarray(W, dtype=np.float32)
    b = np.asarray(b, dtype=np.float32)
    full, _ = run_on_device(x1, x2, W, b, trace=False)
    return full


# revision 26
# speedup vs baseline: 1.2027x; 1.2027x over previous
"""Trainium2 Bass kernel for the BiaffineLayer problem.

Math (per batch b):
  out[l, m, c] = x1[l] @ W1[c] + x2[m] @ W2[c]
              + sum_h x1[l,h] * x2[m,h] * W3[c,h]
              + sum_h |x1[l,h] - x2[m,h]| * W4[c,h] + bias[c]
  shapes: x1, x2 [2, 512, 128]; W [25, 512]; bias [25]; out [2, 512, 512, 25]

Sharding: 8 cores = 2 batches x 4 m-blocks of 128 columns.

Device computes ONLY the pairwise part, in bf16:
  out_dev[l, (m, c)] = sum_h x1t[h,l] * v3[h,(m,c)] + sum_h D_m[h,l] * w4t[h,c]
  with v3[h,(m,c)] = x2t[h,m]*W3T[h,c] (host-computed, DMA'd in) and
  D_m = relu(x1t - x2t[:,m]) (|d| = 2 relu(d) - d; the -d part is rank
  structured and folds into the host epilogue), w4t = (2 W4).T.

Host epilogue adds the rank-structured terms (cheap broadcasts):
  full = out_dev + x1 @ (W1 - W4).T [over l] + (x2 @ (W2 + W4).T + b) [over m]

Per-core schedule (4 blocks of 32 m's, software-pipelined one block ahead):
  - D tiles [128h, 512l] bf16: 7/32 shipped precomputed from host via the
    sync DMA queue; 20/32 on DVE (tensor_scalar sub+max, 2x mode at 341ns);
    5/32 on ACT (Relu with -x2 bias, 641ns).
  - per l-chunk: one PSUM double tile [128, 1024] f32 (two banks: cols
    0:400 and 512:912); 32 t4 matmuls (D_j l-slice stationary, w4t moving;
    start=True only on the first matmul of each bank - start zeroes the
    WHOLE 2KB bank) then 2 t3 matmuls (x1t chunk stationary, v3 slice
    moving, stop=True) - t4-first so the PE needn't wait for the v3 DMA.
  - ACT copies PSUM -> SBUF bf16 [128, 800] (strided 2-bank read), DMA out
    issued from the gpsimd sequencer (its engine is otherwise idle).
"""

import sys

sys.path.insert(0, "/opt/trn_rl_repo")

from contextlib import ExitStack

import numpy as np

import concourse.bass as bass
import concourse.tile as tile
from concourse import bacc, bass_utils, mybir

F32 = mybir.dt.float32
BF16 = mybir.dt.bfloat16

B, L, H, C = 2, 512, 128, 25
MB = 128            # m-block per core
N_CORES = 8
BLK = 32            # m's per psum double-block
N_BLK = MB // BLK   # 4
LCHUNK = 128
N_LC = L // LCHUNK  # 4
HBANK = 16 * C      # 400 used columns per psum bank
PSW = 1024          # psum double tile width (2 banks, f32)

# Engine assignment for the 32 D tiles of each block. The PE consumes js in
# ascending order, so early js go to the fastest producer (DVE at ~263ns
# sustained), ACT tiles (~715ns) are interleaved at its pace, and shipped
# tiles (arriving via batched DMA) fill the tail. Block 0 ships least (its
# tiles are needed before the DMA streams can deliver much); later blocks
# ship more since their deadlines are further out.
SHIP_BLK = {0: tuple(range(26, 32)), 1: tuple(range(20, 32)),
            2: tuple(range(20, 32)), 3: tuple(range(20, 32))}
ACT_BLK = {0: (2, 6, 10, 14, 18), 1: (2, 6, 10, 14),
           2: (2, 6, 10, 14), 3: (2, 6, 10, 14)}
NSHIP = sum(len(v) for v in SHIP_BLK.values())
SHIP_OFF = {}
_off = 0
for _k in range(N_BLK):
    SHIP_OFF[_k] = _off
    _off += len(SHIP_BLK[_k])
# which engine copies each (blk, lc) PSUM tile to SBUF (default ACT)
COPY_ENG = {k: {3: "dve"} for k in range(N_BLK)}


def build_kernel(nc: bass.Bass, repeat: int = 1):
    # x1t and w4t are packed into one tensor so the critical small loads are
    # a single DMA each; tiny multi-descriptor DMAs get starved behind the
    # big ship streams otherwise.
    hbf_d = nc.dram_tensor("hbf", (H, L + C), BF16, kind="ExternalInput").ap()
    x2t_d = nc.dram_tensor("x2t", (H, MB), F32, kind="ExternalInput").ap()
    v3_d = nc.dram_tensor("v3", (H, MB * C), BF16, kind="ExternalInput").ap()
    dship_d = nc.dram_tensor("dship", (H, NSHIP * L), BF16,
                             kind="ExternalInput").ap()
    out = nc.dram_tensor("out", (L, MB * C), BF16, kind="ExternalOutput").ap()

    with tile.TileContext(nc) as tc, ExitStack() as ctx:
      const = ctx.enter_context(tc.tile_pool(name="const", bufs=1))
      ship = ctx.enter_context(tc.tile_pool(name="ship", bufs=N_BLK))
      dpool = ctx.enter_context(tc.tile_pool(name="dpool", bufs=46))
      opool = ctx.enter_context(tc.tile_pool(name="opool", bufs=6))
      psum = ctx.enter_context(tc.tile_pool(name="psum", bufs=4, space="PSUM"))
      for _rep in range(repeat):
        # ---- critical small inputs first on the sync queue: x2t gates all
        # D-gen, hbf (x1t+w4t) gates DVE D-gen and every matmul ----
        x2t = const.tile([H, MB], F32)
        nc.sync.dma_start(x2t[:], x2t_d[:])
        hbf = const.tile([H, L + C], BF16)
        nc.sync.dma_start(hbf[:], hbf_d[:])
        x1t = hbf[:, 0:L]
        w4t = hbf[:, L:L + C]
        # negx2 (the ACT Relu bias) computed on-device: cheaper than a DMA
        negx2 = const.tile([H, MB], F32)
        nc.vector.tensor_scalar_mul(negx2[:], x2t[:], -1.0)

        # shipped D tiles: ONE batched DMA per block (per-tile issue is
        # ~680ns of DGE time, so per-tile DMAs would trickle in too slowly).
        # Block 0 + v3 on sync right after the small loads; blocks 1-3
        # stream on the gpsimd queue.
        ship_blk = {}

        def emit_ship(k, eng):
            w = len(SHIP_BLK[k]) * L
            st = ship.tile([H, w], BF16, tag="ship", name=f"ship_{k}")
            o = SHIP_OFF[k] * L
            eng.dma_start(st[:], dship_d[:, o:o + w])
            ship_blk[k] = st

        # Single input queue in deadline order: two queues steal DMA-engine
        # bandwidth from each other and smear every transfer's completion.
        emit_ship(0, nc.sync)
        v3 = const.tile([H, MB * C], BF16)
        nc.sync.dma_start(v3[:], v3_d[:])
        emit_ship(1, nc.sync)
        emit_ship(2, nc.sync)
        emit_ship(3, nc.sync)

        # ---- main loop, software-pipelined: D-gen one block ahead ----
        all_dts = {}

        def emit_dgen(k):
            ship_j, act_j = SHIP_BLK[k], ACT_BLK[k]
            dts = {}
            for j in range(BLK):
                if j in ship_j:
                    sj = ship_j.index(j)
                    dts[j] = (ship_blk[k], sj * L)
                    continue
                m = k * BLK + j
                dt_ = dpool.tile([H, L], BF16, tag="d")
                if j in act_j:
                    nc.scalar.activation(
                        dt_[:], x1t, mybir.ActivationFunctionType.Relu,
                        bias=negx2[:, m:m + 1], scale=1.0)
                else:
                    nc.vector.tensor_scalar(
                        dt_[:], x1t, x2t[:, m:m + 1], 0.0,
                        op0=mybir.AluOpType.subtract, op1=mybir.AluOpType.max)
                dts[j] = (dt_, 0)
            all_dts[k] = dts

        def emit_compute(k):
            dts = all_dts[k]
            for lc in range(N_LC):
                lsl = slice(lc * LCHUNK, (lc + 1) * LCHUNK)
                ps = psum.tile([LCHUNK, PSW], F32)
                for j in range(BLK):
                    col = j * C if j < 16 else 512 + (j - 16) * C
                    src, off = dts[j]
                    nc.tensor.matmul(
                        ps[:, col:col + C],
                        src[:, off + lc * LCHUNK:off + (lc + 1) * LCHUNK],
                        w4t,
                        start=(j % 16 == 0), stop=False,
                        skip_group_check=True)
                v0 = k * 2 * HBANK
                nc.tensor.matmul(
                    ps[:, 0:HBANK], x1t[:, lsl], v3[:, v0:v0 + HBANK],
                    start=False, stop=True, skip_group_check=True)
                nc.tensor.matmul(
                    ps[:, 512:512 + HBANK], x1t[:, lsl],
                    v3[:, v0 + HBANK:v0 + 2 * HBANK],
                    start=False, stop=True, skip_group_check=True)
                o_sb = opool.tile([LCHUNK, 2 * HBANK], BF16)
                ps_v = ps[:].rearrange("p (b c) -> p b c", b=2)[:, :, 0:HBANK]
                o_v = o_sb[:].rearrange("p (b c) -> p b c", b=2)
                ceng = COPY_ENG.get(k, {}).get(lc, "act")
                if ceng == "dve":
                    nc.vector.tensor_copy(o_v, ps_v)
                else:
                    nc.scalar.copy(o_v, ps_v)
                dma_eng = nc.gpsimd if k < 2 else nc.sync
                dma_eng.dma_start(
                    out[lsl, k * 2 * HBANK:(k + 1) * 2 * HBANK], o_sb[:])

        for k in range(N_BLK + 1):
            if k < N_BLK:
                emit_dgen(k)
            if k >= 1:
                emit_compute(k - 1)
    return nc


_COMPILED = {}


def _get_compiled():
    if "nc" not in _COMPILED:
        nc = bacc.Bacc("TRN2", target_bir_lowering=False, debug=False,
                       num_devices=N_CORES)
        build_kernel(nc)
        nc.compile()
        _COMPILED["nc"] = nc
    return _COMPILED["nc"]


def make_in_maps(x1, x2, W, b):
    W1, W2, W3, W4 = (W[:, 0:H], W[:, H:2 * H], W[:, 2 * H:3 * H],
                      W[:, 3 * H:4 * H])
    bf = np.dtype("bfloat16") if hasattr(np, "bfloat16") else None
    import jax.numpy as jnp  # bf16 conversion helper

    def to_bf16(a):
        return np.asarray(jnp.asarray(a, dtype=jnp.bfloat16))

    w4t = (2.0 * W4).T.astype(np.float32)                # [H, C]
    w3t = np.ascontiguousarray(W3.T, dtype=np.float32)   # [H, C]
    in_maps = []
    for cid in range(N_CORES):
        bb, mblk = cid // 4, cid % 4
        m0 = mblk * MB
        x1t = np.ascontiguousarray(x1[bb].T, dtype=np.float32)   # [H, L]
        x2t = np.ascontiguousarray(x2[bb, m0:m0 + MB].T, dtype=np.float32)
        v3 = (x2t[:, :, None] * w3t[:, None, :]).reshape(H, MB * C)
        ship_ms = [k * BLK + j for k in range(N_BLK) for j in SHIP_BLK[k]]
        dship = np.maximum(
            x1t[None, :, :] - x2t.T[ship_ms][:, :, None], 0.0)  # [NSHIP,H,L]
        dship = dship.transpose(1, 0, 2).reshape(H, NSHIP * L)
        in_maps.append({
            "hbf": to_bf16(np.concatenate([x1t, w4t], axis=1)),
            "x2t": x2t,
            "v3": to_bf16(v3),
            "dship": to_bf16(dship),
        })
    return in_maps


def run_on_device(x1, x2, W, b, trace=False, trace_kwargs=None):
    nc = _get_compiled()
    in_maps = make_in_maps(x1, x2, W, b)
    res = bass_utils.run_bass_kernel_spmd(
        nc, in_maps, core_ids=list(range(N_CORES)), trace=trace,
        **(trace_kwargs or {}))
    W1, W2, W4 = W[:, 0:H], W[:, H:2 * H], W[:, 3 * H:4 * H]
    t1 = x1 @ (W1 - W4).T                    # [B, L, C]
    t2 = x2 @ (W2 + W4).T + b                # [B, L, C]
    full = np.empty((B, L, L, C), dtype=np.float32)
    for cid in range(N_CORES):
        bb, mblk = cid // 4, cid % 4
        m0 = mblk * MB
        blkout = np.asarray(res.results[cid]["out"]).astype(
            np.float32).reshape(L, MB, C)
        blkout += t1[bb][:, None, :]
        blkout += t2[bb, m0:m0 + MB][None, :, :]
        full[bb, :, m0:m0 + MB, :] = blkout
    return full, res


def kernel(x1, x2, W, b):
    x1 = np.asarray(x1, dtype=np.float32)
    x2 = np.asarray(x2, dtype=np.float32)
    W = np.asarray(W, dtype=np.float32)
    b = np.asarray(b, dtype=np.float32)
    full, _ = run_on_device(x1, x2, W, b, trace=False)
    return full
